# revision 19
# baseline (speedup 1.0000x reference)
"""GCN (3-layer, symmetric-norm) on 8 Trainium2 NeuronCores.

Graph/data parallel via dst-vertex cut: each core owns 25000 destination
nodes and their incident edges. Nodes are relabeled into a class-sorted
"pi" order (grouped by padded-degree class, round-robin over 128
partitions) so the per-destination segmented sum becomes regular strided
DVE adds. x[src] gathers run as big dma_gather (SWDGE) ops fetching one
256B unit (8 fp32 rows) per slot from an AllGathered DRAM table; the
wanted row is selected on DVE with an is_equal phase mask and a 3-level
fold (64 -> 8 floats). Self-loop terms never enter the slot/gather path: the table
row of a destination lives in the same partition as its slots, so the
self contribution is one aligned DVE add at the compacted stage.
dinv (with phantom rows zeroed) is computed on host, which also makes
phantom table rows exactly zero — pad slots gather a phantom row and
contribute nothing, removing the indicator mask entirely.
"""
import numpy as np

N = 200000
NCORE = 8
SH = N // NCORE  # 25000 dst nodes per core
FD, HD = 8, 32
B = 48  # gather slots per loop body


def _plan_classes(cnt, max_deg):
    """DP: partition degrees 1..max into classes [lo..hi] (segment size hi),
    minimizing real slot padding + 128-alignment phantom slots. Degree-0
    nodes (no non-loop in-edges) get a dedicated zero-width class."""
    tot = cnt.sum(axis=0)
    INF = float("inf")
    dp = [INF] * (max_deg + 2)
    choice = [0] * (max_deg + 2)
    dp[1] = 0.0
    for lo in range(1, max_deg + 1):
        if dp[lo] == INF:
            continue
        for hi in range(lo, max_deg + 1):
            pad = sum(int(tot[d]) * (hi - d) for d in range(lo, hi + 1))
            phantom = NCORE * 64.0 * hi
            c = dp[lo] + pad + phantom
            if c < dp[hi + 1]:
                dp[hi + 1] = c
                choice[hi + 1] = lo
    classes = []
    hi = max_deg + 1
    while hi > 1:
        lo = choice[hi]
        classes.append((lo, hi - 1))
        hi = lo
    classes.reverse()
    if tot[0] > 0:
        classes = [(0, 0)] + classes
    return classes


def _preprocess(inputs):
    src = np.asarray(inputs["edge_index"][0]).astype(np.int64)
    dst = np.asarray(inputs["edge_index"][1]).astype(np.int64)
    deg = np.bincount(dst, minlength=N)  # slot degree: NO self loop
    deg_full = deg + 1                   # reference deg includes self loop
    max_deg = int(deg.max())

    cnt = np.zeros((NCORE, max_deg + 1), np.int64)
    for c in range(NCORE):
        cnt[c] = np.bincount(deg[c * SH:(c + 1) * SH], minlength=max_deg + 1)

    classes = _plan_classes(cnt, max_deg)
    ncls = len(classes)
    cls_of_deg = np.zeros(max_deg + 1, np.int64)
    seg = np.zeros(ncls, np.int64)
    for i, (lo, hi) in enumerate(classes):
        cls_of_deg[lo:hi + 1] = i
        seg[i] = hi
    cls_cnt = np.zeros((NCORE, ncls), np.int64)
    for c in range(NCORE):
        for i, (lo, hi) in enumerate(classes):
            cls_cnt[c, i] = cnt[c, lo:hi + 1].sum()
    M_cls = cls_cnt.max(axis=0)
    G_cls = (M_cls + 127) // 128
    Gtot = int(G_cls.sum())
    Jn = Gtot * 128
    node_off = np.concatenate([[0], np.cumsum(G_cls)]) * 128
    col_off = np.concatenate([[0], np.cumsum(G_cls * seg)])
    J = int(col_off[-1])
    Jpad = (J + B - 1) // B * B

    pirow = np.zeros(N, np.int64)
    rank_in_core = np.zeros(N, np.int64)
    pad_row = np.zeros(NCORE, np.int64)  # a guaranteed-phantom pirow per core
    for c in range(NCORE):
        nodes = np.arange(c * SH, (c + 1) * SH)
        cl = cls_of_deg[deg[nodes]]
        order = np.lexsort((nodes, cl))
        snodes = nodes[order]
        scl = cl[order]
        ranks = np.zeros(SH, np.int64)
        assigned = np.zeros(Jn, bool)
        for i in range(ncls):
            m = scl == i
            r = node_off[i] + np.arange(int(m.sum()))
            ranks[m] = r
            assigned[r] = True
        rank_in_core[snodes] = ranks
        pirow[snodes] = c * Jn + ranks
        pad_row[c] = c * Jn + int(np.argmin(assigned))  # first phantom rank

    idx_g = np.zeros((NCORE, 128, Jpad), np.int32)
    idx_g[:] = pad_row[:, None, None]
    order_e = np.lexsort((src, dst))
    src_s, dst_s = src[order_e], dst[order_e]
    estart = np.searchsorted(dst_s, np.arange(N))
    e_core = dst_s // SH
    e_rank = rank_in_core[dst_s]
    e_cls = cls_of_deg[deg[dst_s]]
    e_p = e_rank % 128
    e_g = (e_rank - node_off[e_cls]) // 128
    e_col = col_off[e_cls] + e_g * seg[e_cls] + \
        (np.arange(len(src_s)) - estart[dst_s])
    idx_g[e_core, e_p, e_col] = pirow[src_s].astype(np.int32)

    # host dinv in pi layout; phantom rows stay 0 -> phantom table rows are 0
    dinv = 1.0 / np.sqrt(deg_full.astype(np.float64))
    dinv_pi = np.zeros((NCORE, 128, Gtot), np.float32)
    rows_c = pirow % Jn
    cores_n = pirow // Jn
    dinv_pi[cores_n, rows_c % 128, rows_c // 128] = dinv

    # unified typed features, pi-ordered; phantom rows zero
    featU = np.zeros((NCORE, Jn, 50), np.float32)
    t_ev, t_cs, t_tr = 100000, 160000, 199999
    nodes = np.arange(N)
    for lo, hi, key, fo, oh in ((0, t_ev, "ev_features", 0, 46),
                                (t_ev, t_cs, "cs_features", 16, 47),
                                (t_cs, t_tr, "tr_features", 28, 48),
                                (t_tr, N, "env_features", 36, 49)):
        feat = np.asarray(inputs[key], np.float32)
        nn = nodes[lo:hi]
        featU[cores_n[nn], rows_c[nn], fo:fo + feat.shape[1]] = feat
        featU[cores_n[nn], rows_c[nn], oh] = 1.0

    Wcomb = np.zeros((50, FD), np.float32)
    Wcomb[0:16] = np.asarray(inputs["W_ev"], np.float32)
    Wcomb[16:28] = np.asarray(inputs["W_cs"], np.float32)
    Wcomb[28:36] = np.asarray(inputs["W_tr"], np.float32)
    Wcomb[36:46] = np.asarray(inputs["W_env"], np.float32)
    Wcomb[46] = np.asarray(inputs["b_ev"], np.float32)
    Wcomb[47] = np.asarray(inputs["b_cs"], np.float32)
    Wcomb[48] = np.asarray(inputs["b_tr"], np.float32)
    Wcomb[49] = np.asarray(inputs["b_env"], np.float32)

    plan = dict(classes=classes, seg=seg, G_cls=G_cls, Gtot=Gtot, Jn=Jn,
                J=J, Jpad=Jpad, col_off=col_off, node_off=node_off)
    return plan, idx_g, dinv_pi, featU, Wcomb, pirow


def _build_program(plan):
    import concourse.bacc as bacc
    import concourse.tile as tile
    import concourse.mybir as mybir
    import concourse.bass as bass

    Gtot, Jn, Jpad = plan["Gtot"], plan["Jn"], plan["Jpad"]
    classes, seg = plan["classes"], plan["seg"]
    G_cls, col_off, node_off = plan["G_cls"], plan["col_off"], plan["node_off"]
    dt = mybir.dt
    AF = mybir.ActivationFunctionType

    CC = 24  # slab columns per gather chunk (3072 units per dma_gather)
    NCHG = Jpad // CC

    nc = bacc.Bacc("TRN2", target_bir_lowering=False, debug=False,
                   num_devices=NCORE, dynamic_dma_scratch_size=24576)
    featUT_in = nc.dram_tensor("featUT", [50, Jn], dt.float32,
                               kind="ExternalInput").ap()
    idxu_in = nc.dram_tensor("idxu", [128, Jpad * 8], dt.int16,
                             kind="ExternalInput").ap()
    phase_in = nc.dram_tensor("phasep", [128, Jpad], dt.float32,
                              kind="ExternalInput").ap()
    iota_in = nc.dram_tensor("iota8", [128, 8], dt.float32,
                             kind="ExternalInput").ap()
    dinv_in = nc.dram_tensor("dinvp", [128, Gtot], dt.float32,
                             kind="ExternalInput").ap()
    wcomb_in = nc.dram_tensor("wcomb", [50, FD], dt.float32,
                              kind="ExternalInput").ap()
    NW = 8 * 32 + 32 + 32 * 8 + 8 + 8 * 8 + 8
    wrows_in = nc.dram_tensor("wrows", [128, NW],
                              dt.float32, kind="ExternalInput").ap()
    out_d = nc.dram_tensor("outv", [Jn], dt.float32, kind="ExternalOutput").ap()

    with tile.TileContext(nc) as tc:
        with tc.tile_pool(name="sbuf", bufs=1) as pool, \
             tc.tile_pool(name="fslab", bufs=2) as fpool, \
             tc.tile_pool(name="stage", bufs=5) as stage, \
             tc.tile_pool(name="psum", bufs=8, space="PSUM") as psum, \
             tc.tile_pool(name="dram", bufs=1, space="DRAM") as dram:

            table0 = dram.tile([NCORE * Jn, FD], dt.float32,
                               addr_space="Shared", tag="tbl0")
            table1 = dram.tile([NCORE * Jn, FD], dt.float32,
                               addr_space="Shared", tag="tbl1")
            table2 = dram.tile([NCORE * Jn, FD], dt.float32,
                               addr_space="Shared", tag="tbl2")
            shard0 = dram.tile([Jn, FD], dt.float32, tag="shd0")
            shard1 = dram.tile([Jn, FD], dt.float32, tag="shd1")
            shard2 = dram.tile([Jn, FD], dt.float32, tag="shd2")
            tables = [table0, table1, table2]
            shards = [shard0, shard1, shard2]

            iota_t = pool.tile([128, 8], dt.float32)
            nc.sync.dma_start(out=iota_t[:], in_=iota_in[:])
            slab = pool.tile([128, Jpad * FD], dt.float32)
            xcur = pool.tile([128, Gtot * HD], dt.float32)
            agg = pool.tile([128, Gtot * FD], dt.float32)
            yloc = pool.tile([128, Gtot * FD], dt.float32)
            dinv = pool.tile([128, Gtot], dt.float32)
            fin = pool.tile([128, Gtot], dt.float32)
            wc = pool.tile([50, FD], dt.float32)
            wr = pool.tile([128, NW], dt.float32)
            nc.sync.dma_start(out=dinv[:], in_=dinv_in[:])
            nc.sync.dma_start(out=wc[:], in_=wcomb_in[:])
            nc.sync.dma_start(out=wr[:], in_=wrows_in[:])
            cW0 = wr[:, 0:256]
            cb0 = wr[:, 256:288]
            cW1 = wr[:, 288:544]
            cb1 = wr[:, 544:552]
            cW2 = wr[:, 552:616]
            cb2 = wr[:, 616:617]

            # ---------------- embed: x0 = relu(featU @ Wcomb) ----------------
            CH_PER_SLAB = 4
            for sb in range(0, Gtot, CH_PER_SLAB):
                nch = min(CH_PER_SLAB, Gtot - sb)
                fsl = fpool.tile([50, CH_PER_SLAB * 128], dt.float32, tag="fsl")
                nc.sync.dma_start(
                    out=fsl[:, :nch * 128],
                    in_=featUT_in[:, sb * 128: (sb + nch) * 128])
                for c2 in range(nch):
                    ch = sb + c2
                    pt = psum.tile([128, FD], dt.float32, tag="emb")
                    nc.tensor.matmul(out=pt[:],
                                     lhsT=fsl[:, c2 * 128:(c2 + 1) * 128],
                                     rhs=wc[:], start=True, stop=True)
                    nc.scalar.activation(out=agg[:, ch * FD:(ch + 1) * FD],
                                         in_=pt[:], func=AF.Relu)

            # ---------------- class-structured segmented reduction ----------
            def reduce_one_class(buf, W, i):
                s = int(seg[i])
                if s == 0:
                    return
                ng = int(G_cls[i])
                co = int(col_off[i])
                reg = buf[:, co * W:(co + ng * s) * W].rearrange(
                    "p (g c) -> p g c", g=ng)
                w = s
                while w > 1:
                    if w % 2 == 1:
                        nc.vector.tensor_add(
                            out=reg[:, :, 0:W], in0=reg[:, :, 0:W],
                            in1=reg[:, :, (w - 1) * W:w * W])
                        w -= 1
                    h = w // 2
                    if h > 0 and w > 1:
                        nc.vector.tensor_add(
                            out=reg[:, :, 0:h * W], in0=reg[:, :, 0:h * W],
                            in1=reg[:, :, h * W:2 * h * W])
                    w = h

            def compact_classes(buf, W, out):
                for i in range(len(classes)):
                    s = int(seg[i])
                    ng = int(G_cls[i])
                    no = int(node_off[i]) // 128
                    if s == 0:
                        nc.vector.memset(out[:, no * W:(no + ng) * W], 0.0)
                        continue
                    dst3 = out[:, no * W:(no + ng) * W].rearrange(
                        "p (g c) -> p g c", g=ng)
                    co = int(col_off[i])
                    src3 = buf[:, co * W:(co + ng * s) * W].rearrange(
                        "p (g c) -> p g c", g=ng)[:, :, 0:W]
                    nc.vector.tensor_copy(out=dst3, in_=src3)

            def add_self(region, W):
                # self-loop term: + yloc (same (p, g) alignment as compact)
                r3 = region.rearrange("p (g w) -> p g w", w=W)
                y3 = yloc[:, :Gtot * W].rearrange("p (g w) -> p g w", w=W)
                nc.vector.tensor_add(out=r3, in0=r3, in1=y3)

            def scale_by_dinv(region, W):
                d3 = dinv[:].rearrange("p (g o) -> p g o", o=1).to_broadcast(
                    [128, Gtot, W])
                s3 = region.rearrange("p (g w) -> p g w", w=W)
                nc.vector.tensor_mul(out=s3, in0=s3, in1=d3)

            def save_yloc(region):
                nc.vector.tensor_copy(out=yloc[:], in_=region)

            def writeback_allgather(region, li):
                shard_ap = shards[li][:].rearrange("(g p) f -> p g f", p=128)
                nc.sync.dma_start(
                    out=shard_ap, in_=region.rearrange("p (g f) -> p g f", f=FD))
                nc.gpsimd.collective_compute(
                    "AllGather", mybir.AluOpType.bypass,
                    replica_groups=[list(range(NCORE))],
                    ins=[shards[li].opt()], outs=[tables[li].opt()])

            def emit_gather(li):
                # gather 256B units (8 fp32 rows); select the wanted row per
                # slot with an is_equal phase mask, then fold 64 -> 8 floats
                tbl_u = tables[li][:].rearrange("(u e) f -> u (e f)", e=8)
                nxt = [0]

                def drain_reduces(done_cols):
                    # emit reduce trees for classes whose slab columns are
                    # fully gathered, so they overlap later gather chunks
                    while nxt[0] < len(classes) and \
                            int(col_off[nxt[0] + 1]) <= done_cols:
                        reduce_one_class(slab, FD, nxt[0])
                        nxt[0] += 1
                for i in range(NCHG):
                    ixc = stage.tile([128, CC * 8], dt.int16, tag="ix")
                    nc.sync.dma_start(
                        out=ixc[:], in_=idxu_in[:, i * CC * 8:(i + 1) * CC * 8])
                    ob = stage.tile([128, CC * 64], dt.float32, tag="ob")
                    nc.gpsimd.dma_gather(
                        out_ap=ob[:].rearrange("p (c e) -> p c e", e=64),
                        in_ap=tbl_u,
                        idxs_ap=ixc[:],
                        num_idxs=128 * CC,
                        num_idxs_reg=128 * CC,
                        elem_size=64,
                        single_packet=False)
                    phc = stage.tile([128, CC], dt.float32, tag="ph")
                    nc.sync.dma_start(
                        out=phc[:], in_=phase_in[:, i * CC:(i + 1) * CC])
                    m = stage.tile([128, CC * 8], dt.float32, tag="m")
                    m3 = m[:].rearrange("p (c r) -> p c r", r=8)
                    ph3 = phc[:].rearrange(
                        "p (c o) -> p c o", o=1).to_broadcast([128, CC, 8])
                    io3 = iota_t[:].rearrange("p (o r) -> p o r", o=1).to_broadcast(
                        [128, CC, 8])
                    nc.vector.tensor_tensor(out=m3, in0=ph3, in1=io3,
                                            op=mybir.AluOpType.is_equal)
                    ob4 = ob[:].rearrange("p (c r f) -> p c r f", r=8, f=8)
                    m4 = m[:].rearrange("p (c r o) -> p c r o", r=8, o=1).to_broadcast(
                        [128, CC, 8, 8])
                    nc.vector.tensor_mul(out=ob4, in0=ob4, in1=m4)
                    v3 = ob[:].rearrange("p (c x) -> p c x", x=64)
                    nc.vector.tensor_add(out=v3[:, :, 0:32], in0=v3[:, :, 0:32],
                                         in1=v3[:, :, 32:64])
                    nc.vector.tensor_add(out=v3[:, :, 0:16], in0=v3[:, :, 0:16],
                                         in1=v3[:, :, 16:32])
                    nc.vector.tensor_add(out=v3[:, :, 0:8], in0=v3[:, :, 0:8],
                                         in1=v3[:, :, 8:16])
                    nc.vector.tensor_copy(
                        out=slab[:, i * CC * FD:(i + 1) * CC * FD],
                        in_=v3[:, :, 0:8])
                    drain_reduces((i + 1) * CC)
                drain_reduces(Jpad)

            def dve_matmul(src_region, Fin, Fout, wap, out_region):
                o3 = out_region.rearrange("p (g w) -> p g w", w=Fout)
                t3 = slab[:, :Gtot * Fout].rearrange("p (g w) -> p g w", w=Fout)
                s3 = src_region.rearrange("p (g w) -> p g w", w=Fin)
                for fi in range(Fin):
                    sb = s3[:, :, fi:fi + 1].to_broadcast([128, Gtot, Fout])
                    wrow = wap[:, fi * Fout:(fi + 1) * Fout].rearrange(
                        "p (o w) -> p o w", o=1).to_broadcast([128, Gtot, Fout])
                    if fi == 0:
                        nc.vector.tensor_mul(out=o3, in0=sb, in1=wrow)
                    else:
                        nc.vector.tensor_mul(out=t3, in0=sb, in1=wrow)
                        nc.vector.tensor_add(out=o3, in0=o3, in1=t3)

            def add_bias_relu(region, W, bap, relu=True):
                r3 = region.rearrange("p (g w) -> p g w", w=W)
                bb = bap.rearrange("p (o w) -> p o w", o=1).to_broadcast([128, Gtot, W])
                nc.vector.tensor_add(out=r3, in0=r3, in1=bb)
                if relu:
                    nc.vector.tensor_relu(out=r3, in_=r3)

            # y0 = dinv * x0 -> table (phantom rows become 0 via dinv=0)
            x0r = agg[:, :Gtot * FD]
            scale_by_dinv(x0r, FD)
            save_yloc(x0r)
            writeback_allgather(x0r, 0)

            # ---------------- Layer 0 ----------------
            emit_gather(0)
            compact_classes(slab, FD, agg)
            aggr = agg[:, :Gtot * FD]
            add_self(aggr, FD)
            scale_by_dinv(aggr, FD)
            x1r = xcur[:, :Gtot * HD]
            dve_matmul(aggr, FD, HD, cW0, x1r)
            add_bias_relu(x1r, HD, cb0)
            t1r = agg[:, :Gtot * FD]
            dve_matmul(x1r, HD, FD, cW1, t1r)
            scale_by_dinv(t1r, FD)
            save_yloc(t1r)
            writeback_allgather(t1r, 1)

            # ---------------- Layer 1 ----------------
            emit_gather(1)
            compact_classes(slab, FD, agg)
            aggr = agg[:, :Gtot * FD]
            add_self(aggr, FD)
            scale_by_dinv(aggr, FD)
            add_bias_relu(aggr, FD, cb1)          # x2 = relu(dinv*agg + b1)
            t2r = xcur[:, :Gtot * FD]
            dve_matmul(aggr, FD, FD, cW2, t2r)    # x2 @ W2 (col 0)
            scale_by_dinv(t2r, FD)
            save_yloc(t2r)
            writeback_allgather(t2r, 2)

            # ---------------- Layer 2 ----------------
            emit_gather(2)
            compact_classes(slab, FD, agg)
            aggr3 = agg[:, :Gtot * FD].rearrange("p (g w) -> p g w", w=FD)
            yl3 = yloc[:, :Gtot * FD].rearrange("p (g w) -> p g w", w=FD)
            nc.vector.tensor_add(out=aggr3[:, :, 0:1], in0=aggr3[:, :, 0:1],
                                 in1=yl3[:, :, 0:1])
            nc.vector.tensor_copy(out=fin[:].rearrange("p (g o) -> p g o", o=1),
                                  in_=aggr3[:, :, 0:1])
            nc.vector.tensor_mul(out=fin[:], in0=fin[:], in1=dinv[:])
            nc.scalar.activation(out=fin[:], in_=fin[:], func=AF.Tanh,
                                 bias=cb2)
            nc.sync.dma_start(out=out_d.rearrange("(g p) -> p g", p=128),
                              in_=fin[:])
    nc.compile()
    return nc


def kernel(**inputs):
    from concourse.bass_utils import run_bass_kernel_spmd

    plan, idx_g, dinv_pi, featU, Wcomb, pirow = _preprocess(inputs)
    W0 = np.asarray(inputs["W0"], np.float32)
    b0 = np.asarray(inputs["b0"], np.float32)
    W1 = np.asarray(inputs["W1"], np.float32)
    b1 = np.asarray(inputs["b1"], np.float32)
    W2 = np.asarray(inputs["W2"], np.float32)
    b2 = np.asarray(inputs["b2"], np.float32)

    wrows = np.concatenate([W0.reshape(-1), b0, W1.reshape(-1), b1,
                            np.pad(W2[:, 0:1], ((0, 0), (0, 7))).reshape(-1),
                            np.pad(b2, (0, 7))])
    wrows_t = np.tile(wrows[None, :], (128, 1)).astype(np.float32)

    nc = _build_program(plan)

    # unit/phase encoding for the 256B-unit dma_gather path
    CC = 24
    Jpad = plan["Jpad"]
    nch = Jpad // CC
    unit = (idx_g // 8).astype(np.int16)
    phase = (idx_g % 8).astype(np.float32)
    iota8 = np.tile(np.arange(8, dtype=np.float32), (128, 1))
    idxw = np.zeros((NCORE, 128, Jpad * 8), np.int16)
    for c in range(NCORE):
        a = unit[c].reshape(128, nch, CC).transpose(1, 2, 0)  # i, c, p
        flat = a.reshape(nch, CC * 128)        # j = c*128 + p
        w = flat.reshape(nch, CC * 8, 16).transpose(0, 2, 1)  # i, q, s
        w16 = w.transpose(1, 0, 2).reshape(16, Jpad * 8)
        idxw[c] = np.tile(w16, (8, 1))

    in_maps = []
    for c in range(NCORE):
        in_maps.append({
            "featUT": np.ascontiguousarray(featU[c].T),
            "idxu": idxw[c],
            "phasep": phase[c],
            "iota8": iota8,
            "dinvp": dinv_pi[c],
            "wcomb": Wcomb,
            "wrows": wrows_t,
        })
    trace = False
    try:
        from antenv.axon_hooks import get_axon_ntff_profile_hook
        trace = get_axon_ntff_profile_hook() is not None
    except Exception:
        trace = False
    res = run_bass_kernel_spmd(nc, in_maps, list(range(NCORE)), trace=trace)
    global LAST_EXEC_NS
    LAST_EXEC_NS = res.exec_time_ns
    allv = np.concatenate([res.results[c]["outv"] for c in range(NCORE)])
    return allv[pirow].astype(np.float32)
